# revision 26
# baseline (speedup 1.0000x reference)
"""Trainium2 Bass kernel for nn_SCTConv (scattering + GCN attention network).

Sharding: data-parallel over batch B=8 across 8 NeuronCores (one graph per
core), params replicated, no collectives.

v2: transposed-chain fp8 DoubleRow design.

Host-side prep (free):
  - adjM = fp8(A * g[col]) in DoubleRow moving layout [p, j, i, r] where
    column c = j*256 + i*128 + p, g = 0.5*dinv*2^12 (~1.0): the per-column
    random-walk normalization is folded into the fp8 adjacency.
  - u0: pass-1 stationary features fp8 [p, block, 128] = [Q8(X) | Q8(X*c_h)],
    c_h = dsq*2^EH/g.  xT, ds-replicated tiles, MLP weights.

Device (per core, N=4096 nodes, F=64 feats):
  Each pass k streams the resident 16MB fp8 adjacency as the DoubleRow
  MOVING operand (K=256 contraction per instruction, 2x bf16 FLOPs, measured
  562cyc per 512-row matmul) against a tiny stationary feature tile u_k
  (LDW ~25cyc, hidden).  Output lands transposed [64f, 4096n] in PSUM,
  8 chunks of 512 columns.
  - p-state kept scaled: Pt_k = 2^k p_k, so the update is one DVE op
    Pt += 2^-11 * psum (reads PSUM directly) and u_{k+1} = Q8(Pt) is a pure
    ACT copy -> bf16, PE-transposed back to natural fp8 stationary layout.
  - passes 1-2 carry the GCN diffusion chain in stationary cols 64-127
    (psum rows 64-127): 10 logical passes in 8 streams.
  - pass 1 streams behind the adjacency DMA (j-outer, arrival-gated).
  - wavelets fold into single stt ops: s1=2^-1(2X-Pt1) via (xT - 2^-11 ps),
    s2=2^-2(Pt1 - 2^-11 ps), s3=2^-4(4 snap - Pt4), s4=2^-8(16 snap - Pt8);
    |.|*2^-e fused into ACT Abs scale.
  - branch scores e_k = sum_f relu(B_k).a2 as PE matmuls: stationary bf16
    branch block [64,128] x a2 [64,1] -> natural [128,1] slices.  The shared
    relu(X).a1 term cancels in the k-softmax and is dropped entirely.
  - attention accumulated incrementally: wk = exp(e_k) (no max-sub, |e|<~2),
    D += wk, N += wk*B_k as each branch is born; h' = (N/D)/6 at the end.
  - final: h' transpose, W1 (bf16) + Lrelu+b1 on ACT, W2 + b2 + Lrelu, DMA.
"""

import os
import sys
from contextlib import ExitStack

import numpy as np

for _p in ("/opt/trn_rl_repo", "/root/.axon_site/_ro/trn_rl_repo"):
    if os.path.isdir(_p) and _p not in sys.path:
        sys.path.append(_p)

import ml_dtypes
import concourse.bass as bass
import concourse.tile as tile
from concourse import mybir
from concourse.bass_utils import run_bass_kernel_spmd
from concourse.masks import make_identity

N = 4096
F = 64
NCORES = 8
P = 128
NB = N // P          # 32 node blocks
NJ = N // 256        # 16 contraction pair-groups
NCH = 8              # r chunks of 512
CH = N // NCH        # 512
FP32 = mybir.dt.float32
BF16 = mybir.dt.bfloat16
FP8 = mybir.dt.float8e4
FP8NP = mybir.dt.np(FP8)
BF16NP = mybir.dt.np(BF16)
AX = mybir.AxisListType
OP = mybir.AluOpType
AF = mybir.ActivationFunctionType
DR = mybir.MatmulPerfMode.DoubleRow
LEAKY = 0.01
EH = 5               # h-chain psum scale exponent
SH2 = 6              # extra u_h2 scale


def _legalize_waits(nc, cap: int = 1):
    """Split multi-wait/multi-update instructions for this walrus build.

    The container's walrus rejects instructions carrying more than ~1 sync
    wait ("Too many sync wait commands", CoreV3GenImpl setupSyncWait), but
    Tile emits instructions with many waits.  Block instruction lists are
    live, so hoist excess waits onto standalone InstEventSemaphore
    instructions inserted immediately before (same engine, same position --
    semantically identical), and excess updates onto ones inserted after.
    """
    n = 0
    for f in nc.m.functions:
        for b in f.blocks:
            insts = b.instructions  # live list; insert() persists
            i = 0
            while i < len(insts):
                inst = insts[i]
                si = inst.sync_info
                if si is None:
                    i += 1
                    continue
                waits = list(si.on_wait)
                updates = list(si.on_update)
                changed = False
                if len(waits) > cap:
                    extra, waits = waits[:-cap], waits[-cap:]
                    for w in extra:
                        ev = mybir.InstEventSemaphore(
                            name=f"{inst.name}-ws{n}",
                            engine=inst.engine,
                            ins=[],
                            outs=[],
                            sync_info=mybir.SyncInfo(on_wait=[w], on_update=[]),
                        )
                        n += 1
                        insts.insert(i, ev)
                        i += 1
                    changed = True
                if len(updates) > max(cap, 1):
                    updates, extra_u = updates[: max(cap, 1)], updates[max(cap, 1) :]
                    for u in extra_u:
                        ev = mybir.InstEventSemaphore(
                            name=f"{inst.name}-us{n}",
                            engine=inst.engine,
                            ins=[],
                            outs=[],
                            sync_info=mybir.SyncInfo(on_wait=[], on_update=[u]),
                        )
                        n += 1
                        insts.insert(i + 1, ev)
                    changed = True
                if changed:
                    inst.sync_info = mybir.SyncInfo(on_wait=waits, on_update=updates)
                i += 1
    return n


def build_program(moment: int, legalize: bool = True, debug: bool = False,
                  **_ignored) -> bass.Bass:
    nc = bass.Bass()
    dbg = {}
    if debug:
        for nm, shp, dt in (
            ("d_p1", [F, N], FP32), ("d_h1", [F, N], FP32),
            ("d_u1", [P, NB, P], FP8), ("d_h2", [F, N], FP32),
            ("d_p8", [F, N], FP32), ("d_D", [P, NB], FP32),
            ("d_N", [P, NB, F], FP32),
        ):
            dbg[nm] = nc.declare_dram_parameter(nm, shp, dt, isOutput=True)

    adjm_d = nc.declare_dram_parameter("adjM", [P, NJ, 2, N], FP8, isOutput=False)
    u0_d = nc.declare_dram_parameter("u0", [P, NB, P], FP8, isOutput=False)
    xs_d = nc.declare_dram_parameter("xs", [P, N], FP32, isOutput=False)
    ds_d = nc.declare_dram_parameter("dsT", [P, N], BF16, isOutput=False)
    ch_d = nc.declare_dram_parameter("chT", [P, N], BF16, isOutput=False)
    a2_d = nc.declare_dram_parameter("a2c", [P, 1], BF16, isOutput=False)
    w1t_d = nc.declare_dram_parameter("w1t", [F, F], BF16, isOutput=False)
    w2t_d = nc.declare_dram_parameter("w2t", [F, F], BF16, isOutput=False)
    b1_d = nc.declare_dram_parameter("b1c", [F, 1], FP32, isOutput=False)
    b2_d = nc.declare_dram_parameter("b2b", [P, F], FP32, isOutput=False)
    out_d = nc.declare_dram_parameter("out", [P, NB, F], FP32, isOutput=True)

    with ExitStack() as stack:
        tc = stack.enter_context(tile.TileContext(nc))
        const = stack.enter_context(tc.tile_pool(name="const", bufs=1))
        feat = stack.enter_context(tc.tile_pool(name="feat", bufs=1))

        # ---- persistent state tiles (row-packed pairs; bases annotated) ----
        TS = feat.tile([P, N], FP32)      # 0-63: Pt (scaled p) | 64-127: h
        XS = feat.tile([P, N], FP32)      # 0-63: xT -> snap | 64-127: ds^2 fp32
        DSt = feat.tile([P, N], BF16)     # 0-63: ds^2 rep | 64-127: ds rep
        CU = feat.tile([P, N], BF16)      # 0-63: uTb -> hpT | 64-127: c_h*2^SH2
        SB = feat.tile([P, N], BF16)      # 0-63: A (wavelet/score/l1) | 64-127: B
        ubuf = [feat.tile([P, NB, P], FP8, tag=f"u{i}", name=f"u{i}") for i in range(2)]
        Nacc = feat.tile([P, NB, F], FP32)  # N accum; reused as `ot` in final
        Bnat = feat.tile([P, NB, F], BF16)
        Dacc = feat.tile([P, NB], FP32)
        wk_t = feat.tile([P, NB], BF16)
        rD = feat.tile([P, NB], FP32)
        hsc = feat.tile([P, CH], FP32)

        identb = const.tile([P, P], BF16)
        identf = const.tile([P, P], FP32)
        a2c = const.tile([P, 1], BF16)
        w1t = const.tile([F, F], BF16)
        w2t = const.tile([F, F], BF16)
        b1c = const.tile([F, 1], FP32)
        b2b = const.tile([P, F], FP32)


        with tc.tile_pool(name="adj", bufs=1) as adjp, tc.tile_pool(
            name="psq", bufs=5, space="PSUM"
        ) as psq, tc.tile_pool(name="px", bufs=1, space="PSUM") as px:
            # two HW DGE queues, ~balanced (10MB each, ~425GB/s aggregate):
            #   sync: adjM0, ds, xs, adjM evens   act: u0, ch, adjM odds
            # pass-1 consumes j's in predicted arrival order (see mm_sweep)
            adjM = []
            for j in range(NJ):
                adjM.append(adjp.tile([P, 2, N], FP8, tag=f"a{j}", name=f"adj{j}"))
            nc.sync.dma_start(adjM[0][:], adjm_d[:, 0, :, :])
            nc.scalar.dma_start(ubuf[0][:], u0_d[:])
            nc.sync.dma_start(DSt[:], ds_d[:])
            nc.scalar.dma_start(CU[:], ch_d[:])
            nc.sync.dma_start(XS[:], xs_d[:])
            for j in range(1, NJ):
                eng = nc.scalar if j % 2 == 1 else nc.sync
                eng.dma_start(adjM[j][:], adjm_d[:, j, :, :])
            for t, d in ((a2c, a2_d), (w1t, w1t_d), (w2t, w2t_d),
                         (b1c, b1_d), (b2b, b2_d)):
                nc.scalar.dma_start(t[:], d[:])
            make_identity(nc, identb[:])
            make_identity(nc, identf[:])
            # h pass-1 w-term precomputed during the adjacency DMA window;
            # lives in the ds2b rows which pass-1's h_epi then doesn't need
            nc.gpsimd.tensor_mul(DSt[0:F, :], XS[0:F, :], DSt[0:F, :])

            pT = TS[0:F, :]       # base 0
            hT = TS[F:P, :]       # base 64
            xT = XS[0:F, :]       # base 0, dies after pass 1 -> snap
            snap = XS[0:F, :]
            ds2f = XS[F:P, :]     # base 64, dies after pass 2
            ds2b = DSt[0:F, :]    # base 0 (pass-1 h with xT)
            dsT = DSt[F:P, :]     # base 64
            uTb = CU[0:F, :]      # base 0 (pairs with pT) -> hpT in final
            chB = CU[F:P, :]      # base 64 (pairs with hT)
            SBa = SB[0:F, :]      # base 0
            SBb = SB[F:P, :]      # base 64

            def csl(c):
                return slice(c * CH, (c + 1) * CH)

            def chunk_tile(c, k):
                # chunk psum homes: c5/c6 in px, rest rotate 5 psq bufs.
                # NOTE chunk 7 reuses chunk 0's buffer (6th "q" call), so the
                # emitter must place chunk-0's epilogue before chunk-7's MMs.
                if c == 5:
                    return px.tile([P, CH], FP32, tag="c5", name=f"c5_{k}")
                if c == 6:
                    return px.tile([P, CH], FP32, tag="c6", name=f"c6_{k}")
                return psq.tile([P, CH], FP32, tag="q", name=f"q{k}_{c}")

            def mm_sweep(ps, c, uin, M, k):
                for j in range(NJ):
                    nc.tensor.matmul(
                        ps[0:M, :],
                        uin[:, 2 * j : 2 * j + 2, 0:M],
                        adjM[j][:, :, csl(c)],
                        start=(j == 0),
                        stop=(j == NJ - 1),
                        perf_mode=DR,
                    )

            def run_pass(uin, wide, k, drain, cast, arrival=False,
                         next_head=None, pre_done=None):
                """Emit pass-k matmuls with epilogues interleaved two chunks
                behind (so the chunk-7 -> chunk-0 psum reuse is legal and the
                PE never stalls on epilogue transposes).  drain(ps, c) holds
                the psum readers; cast(c) produces the next stationary."""
                M = P if wide else F
                if arrival:
                    # j-outer over chunks 0-6 (stream behind the DMA); js in
                    # predicted queue-arrival order; then drain(0) frees
                    # chunk-7's buffer before its post-DMA sweep.
                    JORD = [0, 1, 3, 2, 5, 4, 7, 6, 9, 8, 11, 10, 13, 12,
                            15, 14]
                    pss = [chunk_tile(c, k) for c in range(7)]
                    for j in JORD:
                        for c in range(7):
                            nc.tensor.matmul(
                                pss[c][0:M, :],
                                uin[:, 2 * j : 2 * j + 2, 0:M],
                                adjM[j][:, :, csl(c)],
                                start=(j == JORD[0]),
                                stop=(j == JORD[-1]),
                                perf_mode=DR,
                            )
                    drain(pss[0], 0)
                    ps7 = chunk_tile(7, k)
                    mm_sweep(ps7, 7, uin, M, k)
                    cast(0)
                    for c in range(1, 7):
                        drain(pss[c], c)
                        cast(c)
                        if c == 3 and next_head is not None:
                            next_head(0)
                    drain(ps7, 7)
                    cast(7)
                    if next_head is not None:
                        next_head(1)
                else:
                    live = {}
                    for c in range(NCH):
                        if pre_done is not None and c in pre_done:
                            live[c] = pre_done[c]
                        else:
                            ps = chunk_tile(c, k)
                            mm_sweep(ps, c, uin, M, k)
                            live[c] = ps
                        if c - 1 in live:
                            drain(live[c - 1], c - 1)
                            cast(c - 1)
                            live.pop(c - 1)
                    for c in sorted(live):
                        drain(live[c], c)
                        cast(c)

            # per-chunk epilogue pieces ------------------------------------
            def p_update(ps, c):
                # Pt += 2^-11 * psum (rows 0-63)
                nc.vector.scalar_tensor_tensor(
                    pT[:, csl(c)], ps[0:F, :], 2.0 ** -11, pT[:, csl(c)],
                    op0=OP.mult, op1=OP.add,
                )

            def u_cast(c, k):
                # uTb = Pt (bf16), transpose to natural fp8 stationary cols 0-63
                uout = ubuf[k % 2]
                nc.scalar.activation(uTb[:, csl(c)], pT[:, csl(c)], AF.Copy)
                pt = px.tile([P, 8, F], BF16, tag="pt", name=f"pt{k}_{c}")
                for q in range(4):
                    bsl = slice(c * CH + q * P, c * CH + (q + 1) * P)
                    nc.tensor.transpose(
                        pt[:, q, :], uTb[:, bsl], identb[0:F, 0:F]
                    )
                nc.vector.tensor_copy(
                    uout[:, 4 * c : 4 * c + 4, 0:F], pt[:, 0:4, :]
                )

            def uh_cast(c, k):
                # u_h staged bf16 in SBb (base 64) -> natural fp8 cols 64-127
                uout = ubuf[k % 2]
                pt = px.tile([P, 8, F], BF16, tag="pt", name=f"ph{k}_{c}")
                for q in range(4):
                    bsl = slice(c * CH + q * P, c * CH + (q + 1) * P)
                    nc.tensor.transpose(
                        pt[:, 4 + q, :], SBb[:, bsl], identb[F:P, F:P]
                    )
                nc.vector.tensor_copy(
                    uout[:, 4 * c : 4 * c + 4, F:P], pt[:, 4:8, :]
                )

            def h_epi(ps, c, src_prev, ds2src, sc, w_pre=None):
                # h_new = dsT * (2^-sc * psum_h + ds2 * src_prev)   (base 64)
                w = hsc[F:P, :]
                if w_pre is None:
                    nc.gpsimd.tensor_mul(
                        w, src_prev[:, csl(c)], ds2src[:, csl(c)]
                    )
                    nc.vector.scalar_tensor_tensor(
                        w, ps[F:P, :], 2.0 ** -sc, w, op0=OP.mult, op1=OP.add,
                    )
                    nc.gpsimd.tensor_mul(hT[:, csl(c)], w, dsT[:, csl(c)])
                    return
                # pass-1: psum scale on ACT, adds/muls on gp (DVE stays free
                # for the p-chain during the post-DMA tail)
                th = hsc[0:F, :]
                nc.scalar.activation(th, ps[F:P, :], AF.Copy, scale=2.0 ** -sc)
                nc.gpsimd.tensor_add(w, th, w_pre[:, csl(c)])
                nc.gpsimd.tensor_mul(hT[:, csl(c)], w, dsT[:, csl(c)])

            # ---------- chunked branch machinery ----------
            # Branch work rides the NEXT pass's cast slots (chunked), where
            # the psq rotation has slack for the score psum `pe`.  Staging:
            #   s1-t -> SBa (drain1), consumed cast2
            #   s2-t -> CU[F:P] (chB dead after pass 1), consumed cast3
            #   hA:  lrelu -> SBb, relu -> DSt[0:F] (ds2b dead), both drain2,
            #        consumed cast2
            #   hA2: staged chunk-wise right after pass 2, consumed cast3
            #   s3-t -> SBa (drain4), consumed cast4
            #   s4-t -> SBa (drain8), consumed cast8
            def wavelet_abs_chunk(c, src, escale):
                # SBa chunk = |src * escale| ** moment  (src may be SBa)
                if moment == 0:
                    nc.vector.memset(SBa[:, csl(c)], 1.0)
                    return
                nc.scalar.activation(SBa[:, csl(c)], src[:, csl(c)],
                                     AF.Abs, scale=escale)
                if moment == 2:
                    nc.gpsimd.tensor_mul(
                        SBa[:, csl(c)], SBa[:, csl(c)], SBa[:, csl(c)]
                    )
                elif moment > 2:
                    cp = hsc[0:F, :]
                    nc.gpsimd.tensor_copy(cp, SBa[:, csl(c)])
                    for _ in range(moment - 2):
                        nc.gpsimd.tensor_mul(
                            SBa[:, csl(c)], SBa[:, csl(c)], cp
                        )

            def branch_trans_chunk(tag, c, src, base64):
                pt = px.tile([P, 8, F], BF16, tag="pt", name=f"bt{tag}_{c}")
                off = 4 if base64 else 0
                for q in range(4):
                    bsl = slice(c * CH + q * P, c * CH + (q + 1) * P)
                    nc.tensor.transpose(
                        pt[:, off + q, :], src[:, bsl],
                        identb[F:P, F:P] if base64 else identb[0:F, 0:F],
                    )
                nc.scalar.activation(
                    Bnat[:, 4 * c : 4 * c + 4, :], pt[:, off : off + 4, :],
                    AF.Copy,
                )

            def branch_score_chunk(kidx, c, src, base64):
                pe = psq.tile([P, CH], FP32, tag="q",
                              name=f"pe{kidx}_{c}")[:, 0:NB]
                a2v = a2c[F:P, :] if base64 else a2c[0:F, :]
                for b in range(4 * c, 4 * c + 4):
                    bsl = slice(b * P, (b + 1) * P)
                    nc.tensor.matmul(
                        pe[:, b : b + 1], src[:, bsl], a2v,
                        start=True, stop=True,
                    )
                sl = slice(4 * c, 4 * c + 4)
                wkb = wk_t[:, sl, None].broadcast_to([P, 4, F])
                nc.scalar.activation(wk_t[:, sl], pe[:, sl], AF.Exp)
                if kidx == 0:
                    nc.gpsimd.tensor_copy(Dacc[:, sl], wk_t[:, sl])
                    nc.gpsimd.tensor_mul(Nacc[:, sl, :], Bnat[:, sl, :], wkb)
                else:
                    nc.gpsimd.tensor_add(Dacc[:, sl], Dacc[:, sl], wk_t[:, sl])
                    nc.gpsimd.tensor_mul(Bnat[:, sl, :], Bnat[:, sl, :], wkb)
                    nc.gpsimd.tensor_add(
                        Nacc[:, sl, :], Nacc[:, sl, :], Bnat[:, sl, :]
                    )

            def wavelet_chunk(kidx, c, src, escale):
                wavelet_abs_chunk(c, src, escale)
                branch_trans_chunk(kidx, c, SBa, base64=False)
                branch_score_chunk(kidx, c, SBa, base64=False)

            def ha_chunk(kidx, c):
                # staged: lrelu in SBb, relu in DSt[0:F]
                branch_trans_chunk(kidx, c, SBb, base64=True)
                branch_score_chunk(kidx, c, DSt[0:F, :], base64=False)

            # ================= pass 1 (wide, arrival-gated) =================
            def drain1(ps, c):
                # s1-t = xT - 2^-11*ps  (s1 = |0.5 t|^m, taken at cast2)
                nc.vector.scalar_tensor_tensor(
                    SBa[:, csl(c)], ps[0:F, :], -(2.0 ** -11),
                    xT[:, csl(c)], op0=OP.mult, op1=OP.add,
                )
                # Pt1 = X + 2^-11 ps (pT starts uninitialized; X via xT)
                nc.vector.scalar_tensor_tensor(
                    pT[:, csl(c)], ps[0:F, :], 2.0 ** -11, xT[:, csl(c)],
                    op0=OP.mult, op1=OP.add,
                )
                h_epi(ps, c, xT, ds2b, EH, w_pre=DSt[0:F, :])

            def cast1(c):
                u_cast(c, k=1)
                # u_h2 = Q8(h1 * c_h * 2^SH2): stage bf16 in SBb (base 64)
                nc.gpsimd.tensor_mul(
                    SBb[:, csl(c)], hT[:, csl(c)], chB[:, csl(c)]
                )
                uh_cast(c, k=1)

            p2c0 = {}

            def p2_head(step):
                # pass-2's chunk-0 sweep in two halves, interleaved into
                # pass-1's cast pipeline (j-pair 2c needs cast1(c))
                if step == 0:
                    p2c0[0] = chunk_tile(0, 2)
                rng = range(0, 8) if step == 0 else range(8, NJ)
                for j in rng:
                    nc.tensor.matmul(
                        p2c0[0][0:P, :],
                        ubuf[1][:, 2 * j : 2 * j + 2, 0:P],
                        adjM[j][:, :, csl(0)],
                        start=(j == 0),
                        stop=(j == NJ - 1),
                        perf_mode=DR,
                    )

            with nc.named_scope("pass1"):
                run_pass(ubuf[0], wide=True, k=1, drain=drain1, cast=cast1,
                         arrival=True, next_head=p2_head)

            if debug:
                nc.sync.dma_start(dbg["d_p1"][:], pT[:, :])
                nc.sync.dma_start(dbg["d_h1"][:], hT[:, :])
                nc.sync.dma_start(dbg["d_u1"][:], ubuf[1][:])

            # ================= pass 2 (wide) =================
            def drain2(ps, c):
                # s2-t = Pt1 - 2^-11*ps -> parked in CU[F:P] until cast3
                nc.vector.scalar_tensor_tensor(
                    CU[F:P, csl(c)], ps[0:F, :], -(2.0 ** -11),
                    pT[:, csl(c)], op0=OP.mult, op1=OP.add,
                )
                p_update(ps, c)
                # hA staging from h1 before h_epi overwrites it
                nc.vector.scalar_tensor_tensor(
                    SBb[:, csl(c)], hT[:, csl(c)], LEAKY, hT[:, csl(c)],
                    op0=OP.mult, op1=OP.max,
                )
                nc.scalar.activation(
                    DSt[0:F, csl(c)], hT[:, csl(c)], AF.Relu
                )
                h_epi(ps, c, hT, ds2f, EH + SH2)

            def cast2(c):
                u_cast(c, k=2)
                wavelet_chunk(0, c, SBa, 0.5)       # s1
                ha_chunk(1, c)                      # hA

            with nc.named_scope("pass2"):
                run_pass(ubuf[1], wide=True, k=2, drain=drain2, cast=cast2,
                         pre_done={0: p2c0[0]})
            if debug:
                nc.sync.dma_start(dbg["d_h2"][:], hT[:, :])

            # hA2 staging (h2 is final; chunk ops so cast3(0) unblocks fast)
            with nc.named_scope("hA2_stage"):
                for c in range(NCH):
                    nc.vector.scalar_tensor_tensor(
                        SBb[:, csl(c)], hT[:, csl(c)], LEAKY, hT[:, csl(c)],
                        op0=OP.mult, op1=OP.max,
                    )
                    nc.scalar.activation(
                        DSt[0:F, csl(c)], hT[:, csl(c)], AF.Relu
                    )

            # ================= passes 3..8 =================
            def make_epi(k):
                def drain(ps, c):
                    if k in (3, 5):  # snapshot Pt2 / Pt4 before update (ACT)
                        nc.scalar.activation(
                            snap[:, csl(c)], pT[:, csl(c)], AF.Copy
                        )
                    p_update(ps, c)
                    if k == 4:   # s3-t = 4*snap - Pt4
                        nc.vector.scalar_tensor_tensor(
                            SBa[:, csl(c)], snap[:, csl(c)], 4.0,
                            pT[:, csl(c)], op0=OP.mult, op1=OP.subtract,
                        )
                    if k == 8:   # s4-t = 16*snap - Pt8
                        nc.vector.scalar_tensor_tensor(
                            SBa[:, csl(c)], snap[:, csl(c)], 16.0,
                            pT[:, csl(c)], op0=OP.mult, op1=OP.subtract,
                        )

                def cast(c):
                    if k < 8:
                        u_cast(c, k=k)
                    if k == 3:
                        wavelet_chunk(3, c, CU[F:P, :], 0.25)  # s2
                        ha_chunk(2, c)                         # hA2
                    if k == 4:
                        wavelet_chunk(4, c, SBa, 2.0 ** -4)
                    if k == 8:
                        wavelet_chunk(5, c, SBa, 2.0 ** -8)
                return drain, cast

            for k in range(3, 9):
                drain, cast = make_epi(k)
                with nc.named_scope(f"pass{k}"):
                    run_pass(ubuf[(k - 1) % 2], wide=False, k=k,
                             drain=drain, cast=cast)
            if debug:
                nc.sync.dma_start(dbg["d_p8"][:], pT[:, :])
                nc.sync.dma_start(dbg["d_D"][:], Dacc[:])
                nc.sync.dma_start(dbg["d_N"][:], Nacc[:])

        # ================= final: h', MLP, out =================
        # 4 groups of 8 node-blocks, each an independent pipeline
        # (recip/hp -> transposes -> l1 -> l2 -> bias+lrelu -> DMA) so group
        # g's DVE/ACT work overlaps group g-1's PE work.
        with nc.named_scope("final"):
            with tc.tile_pool(name="psF", bufs=2, space="PSUM") as psF, tc.tile_pool(
                name="psL", bufs=2, space="PSUM"
            ) as psL, tc.tile_pool(name="psO", bufs=2, space="PSUM") as psO:
                hpT = CU[0:F, :]  # bf16 [64, N] base 0 (pairs with w1t)
                nc.vector.reciprocal(rD[:], Dacc[:])
                for g in range(4):
                    sl8 = slice(g * 8, (g + 1) * 8)
                    shp = [P, 8, F]
                    # hp = (N/6) * rD -> bf16 into Bnat (this 8-block group)
                    nc.vector.scalar_tensor_tensor(
                        Bnat[:, sl8, :], Nacc[:, sl8, :], 1.0 / 6.0,
                        rD[:, sl8, None].broadcast_to(shp),
                        op0=OP.mult, op1=OP.mult,
                    )
                    for b in range(8 * g, 8 * g + 8):
                        pf = psF.tile([F, P], BF16, tag="f", name=f"hp{b}")
                        nc.tensor.transpose(pf[:], Bnat[:, b, :], identb[:])
                        nc.scalar.activation(
                            hpT[:, b * P : (b + 1) * P], pf[:], AF.Copy
                        )
                    for c in (2 * g, 2 * g + 1):
                        pl = psL.tile([F, CH], FP32, tag="l", name=f"l1{c}")
                        nc.tensor.matmul(
                            pl[:], w1t[:], hpT[:, csl(c)],
                            start=True, stop=True,
                        )
                        nc.scalar.activation(
                            SBa[:, csl(c)], pl[:], AF.Lrelu, bias=b1c[:, 0:1],
                            alpha=LEAKY,
                        )
                    po = psO.tile([P, 8, F], FP32, tag="o", name=f"o{g}")
                    for i, b in enumerate(range(8 * g, 8 * g + 8)):
                        nc.tensor.matmul(
                            po[:, i, :], SBa[:, b * P : (b + 1) * P], w2t[:],
                            start=True, stop=True,
                        )
                    nc.vector.tensor_add(
                        Nacc[:, sl8, :], po[:],
                        b2b[:, None, :].broadcast_to(shp),
                    )
                    nc.vector.scalar_tensor_tensor(
                        Nacc[:, sl8, :], Nacc[:, sl8, :], LEAKY,
                        Nacc[:, sl8, :], op0=OP.mult, op1=OP.max,
                    )
                    nc.sync.dma_start(out_d[:, sl8, :], Nacc[:, sl8, :])

    if legalize:
        _legalize_waits(nc)
    return nc


_cache: dict = {}


def _get_program(moment: int) -> bass.Bass:
    if moment not in _cache:
        _cache[moment] = build_program(moment)
    return _cache[moment]


def _make_in_maps(X, adj, W1, b1, W2, b2, a):
    X = np.asarray(X, np.float32)
    adj = np.asarray(adj, np.float32)
    w1tb = np.asarray(W1, np.float32).T.astype(BF16NP)
    w2tb = np.asarray(W2, np.float32).T.astype(BF16NP)
    b1c = np.ascontiguousarray(np.asarray(b1, np.float32).reshape(F, 1))
    b2b = np.ascontiguousarray(
        np.broadcast_to(np.asarray(b2, np.float32).reshape(F), (P, F))
    )
    av = np.asarray(a, np.float32).reshape(2 * F)
    a2c = np.ascontiguousarray(np.concatenate([av[F:], av[F:]]).reshape(P, 1)).astype(BF16NP)
    maps = []
    for cidx in range(NCORES):
        ac = adj[cidx]
        rowsum = ac.sum(axis=1, dtype=np.float64)
        dinv = 1.0 / rowsum
        dsq = 1.0 / np.sqrt(rowsum + 1.0)
        g = (0.5 * dinv * 2.0 ** 12).astype(np.float32)
        adjMf = ac * g[None, :]
        adjM = np.ascontiguousarray(
            adjMf.T.reshape(NJ, 2, P, N).transpose(2, 0, 1, 3)
        ).astype(FP8NP)
        c_h = ((dsq * 2.0 ** EH) / g.astype(np.float64)).astype(np.float32)
        Xc = X[cidx]
        u0 = np.zeros((P, NB, P), FP8NP)
        u0[:, :, 0:F] = Xc.reshape(NB, P, F).transpose(1, 0, 2).astype(FP8NP)
        u0[:, :, F:P] = (
            (Xc * c_h[:, None]).reshape(NB, P, F).transpose(1, 0, 2)
        ).astype(FP8NP)
        xs = np.empty((P, N), np.float32)
        xs[0:F] = Xc.T
        xs[F:P] = np.broadcast_to((dsq * dsq).astype(np.float32), (F, N))
        dsTt = np.empty((P, N), BF16NP)
        dsTt[0:F] = np.broadcast_to(
            (dsq * dsq).astype(np.float32), (F, N)
        ).astype(BF16NP)
        dsTt[F:P] = np.broadcast_to(dsq.astype(np.float32), (F, N)).astype(BF16NP)
        chT = np.zeros((P, N), BF16NP)
        chT[F:P] = np.broadcast_to(
            (c_h * 2.0 ** SH2).astype(np.float32), (F, N)
        ).astype(BF16NP)
        maps.append(
            dict(
                adjM=adjM, u0=u0, xs=xs, dsT=dsTt, chT=chT,
                a2c=a2c, w1t=w1tb, w2t=w2tb, b1c=b1c, b2b=b2b,
            )
        )
    return maps


def run(X, adj, W1, b1, W2, b2, a, moment, trace=False):
    m = int(np.asarray(moment))
    nc = _get_program(m)
    in_maps = _make_in_maps(X, adj, W1, b1, W2, b2, a)
    res = run_bass_kernel_spmd(nc, in_maps, list(range(NCORES)), trace=trace)
    out = np.stack(
        [
            np.asarray(res.results[c]["out"])
            .reshape(P, NB, F)
            .transpose(1, 0, 2)
            .reshape(N, F)
            for c in range(NCORES)
        ],
        axis=0,
    )
    return out.astype(np.float32, copy=False), res


def kernel(X, adj, W1, b1, W2, b2, a, moment):
    out, _ = run(X, adj, W1, b1, W2, b2, a, moment)
    return out
